# revision 7
# baseline (speedup 1.0000x reference)
"""RTRL QuasiLSTM layer kernel for Trainium2 (8 NeuronCores, batch-sharded).

Math: per (b, h) the RTRL state recurrences are *linear* given the gate
sequence:  Z_t = c_t * Z_{t-1} + zf_t * x_t.  Only the final Z/F are outputs,
so  Z_S = (prod c) * Z_0 + sum_t suffix_t * zf_t * x_t  — the big [B,H,D]
accumulation becomes TensorE matmuls (G_b^T @ X_b).  The only sequential part
is the [B,H]-sized nonlinear cell recurrence (256 steps of small DVE/ACT ops).
"""

import sys

if "/opt/trn_rl_repo" not in sys.path:
    sys.path.insert(0, "/opt/trn_rl_repo")

import numpy as np

import concourse.bass as bass
import concourse.tile as tile
from concourse import bacc, mybir
from concourse.bass_utils import run_bass_kernel_spmd

F32 = mybir.dt.float32
OP = mybir.AluOpType
AF = mybir.ActivationFunctionType

S, B, D, H = 256, 32, 512, 512
NCORES = 8
BL = B // NCORES      # batches per core
HH = H // 128         # h partition-tiles
J = HH * BL           # 16 small-tensor columns: j = b*HH + hh, partition = h_lo
DT = D // 128         # d partition-tiles


def _ap(t, offset, dims):
    """Custom AP into a tile: keeps partition dim, replaces free dims."""
    base = t if isinstance(t, bass.AP) else t[:]
    return bass.AP(tensor=base.tensor, offset=base.offset + offset,
                   ap=[list(base.ap[0])] + [list(dd) for dd in dims])


def _dap(t, offset, pdim, dims):
    """Fully explicit AP (DRAM side): pdim = partition-matching [step, count]."""
    base = t if isinstance(t, bass.AP) else t.ap()
    return bass.AP(tensor=base.tensor, offset=base.offset + offset,
                   ap=[list(pdim)] + [list(dd) for dd in dims])


def build_kernel(nc, s_len=S, limit=None):
    SC = s_len // 128          # s-chunks (also K-tiles of the state matmuls)
    assert s_len % 128 == 0
    CH = 64                    # bulk-phase chunk (steps)
    NCH = s_len // CH
    CELC = 32                  # cells output dma chunk (steps)

    io = {}
    io["x"] = nc.dram_tensor("x", [s_len, BL, D], F32, kind="ExternalInput")
    io["xt"] = nc.dram_tensor("xt", [128, DT * s_len * BL], F32,
                              kind="ExternalInput")
    io["wmt"] = nc.dram_tensor("wmt", [128, 2 * DT * 512], F32,
                               kind="ExternalInput")
    io["hp"] = nc.dram_tensor("hp", [128, J], F32, kind="ExternalInput")
    io["z0"] = nc.dram_tensor("z0", [BL, H, D], F32, kind="ExternalInput")
    io["f0"] = nc.dram_tensor("f0", [BL, H, D], F32, kind="ExternalInput")
    for n in ("wz0", "wf0", "bz0", "bf0"):
        io[n] = nc.dram_tensor(n, [128, J], F32, kind="ExternalInput")
    for n in ("wvz", "wvf", "bsz", "bsf"):
        io[n] = nc.dram_tensor(n, [128, HH], F32, kind="ExternalInput")
    io["ident"] = nc.dram_tensor("ident", [128, 128], F32, kind="ExternalInput")

    io["cells"] = nc.dram_tensor("cells", [128, s_len * J], F32,
                                 kind="ExternalOutput")
    io["zout"] = nc.dram_tensor("zout", [BL, H, D], F32, kind="ExternalOutput")
    io["fout"] = nc.dram_tensor("fout", [BL, H, D], F32, kind="ExternalOutput")
    for n in ("wzn", "wfn", "bzn", "bfn"):
        io[n] = nc.dram_tensor(n, [128, J], F32, kind="ExternalOutput")

    with tile.TileContext(nc) as tc:
        _body(nc, tc, io, s_len, SC, CH, NCH, CELC, limit)
    nc.compile()
    return nc


def _body(nc, tc, io, s_len, SC, CH, NCH, CELC, limit=None):
    import contextlib
    ctx = contextlib.ExitStack()
    with ctx:
        big = ctx.enter_context(tc.tile_pool(name="big", bufs=1))
        cons = ctx.enter_context(tc.tile_pool(name="cons", bufs=1))
        sc_p = ctx.enter_context(tc.tile_pool(name="scan", bufs=3))
        b2_p = ctx.enter_context(tc.tile_pool(name="bulk", bufs=2))
        cel_p = ctx.enter_context(tc.tile_pool(name="celp", bufs=3))
        str_p = ctx.enter_context(tc.tile_pool(name="strm", bufs=3))
        out_p = ctx.enter_context(tc.tile_pool(name="outp", bufs=3))
        pp1 = ctx.enter_context(tc.tile_pool(name="pp1", bufs=3, space="PSUM"))
        ppT = ctx.enter_context(tc.tile_pool(name="ppT", bufs=2, space="PSUM"))
        pp5 = ctx.enter_context(tc.tile_pool(name="pp5", bufs=3, space="PSUM"))

        # ------------- big SBUF arrays -------------
        pzfb = big.tile([128, s_len * 2 * J], F32, tag="pzfb")   # pre-acts -> s
        cp = big.tile([128, (s_len + 1) * J], F32, tag="cp")     # cell_t + 1
        zf = big.tile([128, s_len * J], F32, tag="zf")           # zf -> G_z
        fz = big.tile([128, s_len * J], F32, tag="fz")           # fz -> G_f
        cc = big.tile([128, s_len * J], F32, tag="cc")
        sfx = big.tile([128, s_len * J], F32, tag="sfx")
        xnat = big.tile([128, SC * BL * D], F32, tag="xnat")     # [s_lo,(kt,b,d)]
        xT = big.tile([128, DT * s_len * BL], F32, tag="xT")     # [d_lo,(dt,s,b)]
        wmT = big.tile([128, 2 * DT * 512], F32, tag="wmT")      # [d_lo,(g,dt,h)]

        # ------------- const loads -------------
        idn = cons.tile([128, 128], F32, tag="idn")
        nc.sync.dma_start(out=idn, in_=io["ident"][:, :])

        wvz_t = cons.tile([128, HH], F32, tag="wvz")
        wvf_t = cons.tile([128, HH], F32, tag="wvf")
        bsz_t = cons.tile([128, HH], F32, tag="bsz")
        bsf_t = cons.tile([128, HH], F32, tag="bsf")
        for t_, n_ in ((wvz_t, "wvz"), (wvf_t, "wvf"), (bsz_t, "bsz"),
                       (bsf_t, "bsf")):
            nc.sync.dma_start(out=t_, in_=io[n_][:, :])

        def load_bh(dst, name):  # [BL,H] -> [128, J] (col = hh*BL + b)
            nc.sync.dma_start(out=_ap(dst, 0, [[1, J]]),
                              in_=io[name][:, :])

        hp_t = cons.tile([128, J], F32, tag="hp")
        wz0_t = cons.tile([128, J], F32, tag="wz0")
        wf0_t = cons.tile([128, J], F32, tag="wf0")
        bz0_t = cons.tile([128, J], F32, tag="bz0")
        bf0_t = cons.tile([128, J], F32, tag="bf0")
        load_bh(hp_t, "hp")
        load_bh(wz0_t, "wz0")
        load_bh(wf0_t, "wf0")
        load_bh(bz0_t, "bz0")
        load_bh(bf0_t, "bf0")

        # wv2_full [128, 2J]: z-half 2*wv_z (bcast over b), f-half wv_f
        wv2 = cons.tile([128, 2 * J], F32, tag="wv2")
        wvz_r = cons.tile([128, J], F32, tag="wvzr")   # wv_z bcast over b
        wvf_r = cons.tile([128, J], F32, tag="wvfr")
        for hh in range(HH):
            src_z = _ap(wvz_t, hh, [[0, BL]])
            src_f = _ap(wvf_t, hh, [[0, BL]])
            nc.vector.tensor_scalar(_ap(wv2, hh, [[HH, BL]]), src_z,
                                    2.0, None, OP.mult)
            nc.vector.tensor_copy(_ap(wv2, J + hh, [[HH, BL]]), src_f)
            nc.vector.tensor_copy(_ap(wvz_r, hh, [[HH, BL]]), src_z)
            nc.vector.tensor_copy(_ap(wvf_r, hh, [[HH, BL]]), src_f)

        # per-partition bias vectors for the P1 psum->sbuf copies
        bWz = cons.tile([128, HH], F32, tag="bWz")   # 2*(bsz - wvz)
        bWf = cons.tile([128, HH], F32, tag="bWf")   # bsf - wvf
        nc.vector.scalar_tensor_tensor(bWz, wvz_t, -1.0, bsz_t, OP.mult, OP.add)
        nc.vector.tensor_scalar(bWz, bWz, 2.0, None, OP.mult)
        nc.vector.scalar_tensor_tensor(bWf, wvf_t, -1.0, bsf_t, OP.mult, OP.add)

        # CP block 0 = hidden_prev + 1
        nc.vector.tensor_scalar(_ap(cp, 0, [[1, J]]), hp_t, 1.0, None, OP.add)

        # ------------- x / wm loads -------------
        for kt in range(SC):
            nc.sync.dma_start(
                out=_ap(xnat, kt * BL * D, [[1, BL * D]]),
                in_=_dap(io["x"], kt * 128 * BL * D, [BL * D, 128],
                         [[1, BL * D]]))
        nc.sync.dma_start(out=xT[:], in_=io["xt"][:, :])
        nc.sync.dma_start(out=wmT[:], in_=io["wmt"][:, :])

        # ------------- P1: pre-projections -------------
        for g in range(2):
            for hh in range(HH):
                for sc in range(SC):
                    ps = pp1.tile([128, 128 * BL], F32, tag="pp1")
                    for dtt in range(DT):
                        nc.tensor.matmul(
                            ps,
                            _ap(wmT, g * DT * 512 + dtt * 512 + hh * 128,
                                [[1, 128]]),
                            _ap(xT, dtt * s_len * BL + sc * 128 * BL,
                                [[1, 128 * BL]]),
                            start=(dtt == 0), stop=(dtt == DT - 1))
                    nc.scalar.activation(
                        _ap(pzfb, (sc * 128) * 2 * J + g * J + hh,
                            [[2 * J, 128], [HH, BL]]),
                        _ap(ps, 0, [[BL, 128], [1, BL]]),
                        AF.Identity,
                        bias=bWz[:, hh:hh + 1] if g == 0 else bWf[:, hh:hh + 1],
                        scale=2.0 if g == 0 else 1.0)

        if limit == 'p1':
            return
        # ------------- P2a: sequential cell scan -------------
        v2 = [[J, 2], [1, J]]
        for t in range(s_len):
            accw = sc_p.tile([128, 2 * J], F32, tag="accw")
            acc = sc_p.tile([128, 2 * J], F32, tag="acc")
            dt_t = sc_p.tile([128, J], F32, tag="dt")
            mt_t = sc_p.tile([128, J], F32, tag="mt")
            # accw = wv2 * [CP_t | CP_t]
            nc.vector.tensor_tensor(
                _ap(accw, 0, v2), _ap(wv2, 0, v2),
                _ap(cp, t * J, [[0, 2], [1, J]]), OP.mult)
            # acc = accw + pzfb_t
            nc.vector.tensor_tensor(
                _ap(acc, 0, v2), _ap(accw, 0, v2),
                _ap(pzfb, t * 2 * J, v2), OP.add)
            # s = sigmoid(acc) -> overwrite pzfb block t
            nc.scalar.activation(_ap(pzfb, t * 2 * J, v2), _ap(acc, 0, v2),
                                 AF.Sigmoid)
            # d = cell - z = CP_t - 2*sz
            nc.vector.scalar_tensor_tensor(
                dt_t, _ap(pzfb, t * 2 * J, [[1, J]]), -2.0,
                _ap(cp, t * J, [[1, J]]), OP.mult, OP.add)
            # m = f * d
            nc.vector.tensor_tensor(
                mt_t, _ap(pzfb, t * 2 * J + J, [[1, J]]), dt_t, OP.mult)
            # CP_{t+1} = 2*sz + m
            nc.vector.scalar_tensor_tensor(
                _ap(cp, (t + 1) * J, [[1, J]]),
                _ap(pzfb, t * 2 * J, [[1, J]]), 2.0, mt_t, OP.mult, OP.add)

            if (t + 1) % CELC == 0:
                k = (t + 1) // CELC - 1
                ctmp = cel_p.tile([128, CELC * J], F32, tag="ctmp")
                nc.vector.tensor_scalar(
                    ctmp, _ap(cp, (k * CELC + 1) * J, [[1, CELC * J]]),
                    -1.0, None, OP.add)
                nc.sync.dma_start(
                    out=_dap(io["cells"], k * CELC * J,
                             [s_len * J, 128], [[1, CELC * J]]),
                    in_=_ap(ctmp, 0, [[1, CELC * J]]))

        if limit == 'p2a':
            return
        # ------------- P2b: bulk gate math -------------
        for ch in range(NCH):
            b0 = ch * CH
            W = CH * J
            tcv = [[J, CH], [1, J]]
            dims = [[2 * J, CH], [1, J]]
            szv = _ap(pzfb, b0 * 2 * J, dims)
            fv = _ap(pzfb, b0 * 2 * J + J, dims)
            cpv = _ap(cp, b0 * J, tcv)
            zfv = _ap(zf, b0 * J, tcv)
            fzv = _ap(fz, b0 * J, tcv)
            ccv = _ap(cc, b0 * J, tcv)

            t1 = b2_p.tile([128, W], F32, tag="t1")
            t2 = b2_p.tile([128, W], F32, tag="t2")
            t1v = _ap(t1, 0, tcv)
            t2v = _ap(t2, 0, tcv)
            # vz = sz - sz^2 (in t2) ; u = 1-f (in t1) ; zf = 4*vz*u
            nc.scalar.activation(t1v, szv, AF.Square)
            nc.vector.tensor_tensor(t2v, szv, t1v, OP.subtract)
            nc.vector.tensor_scalar(t1v, fv, -1.0, 1.0, OP.mult, OP.add)
            nc.vector.scalar_tensor_tensor(zfv, t2v, 4.0, t1v, OP.mult, OP.mult)
            # vf = f - f^2 (t2) ; d2 = CP - 2sz (t1) ; fz = d2*vf
            nc.scalar.activation(t2v, fv, AF.Square)
            nc.vector.tensor_tensor(t2v, fv, t2v, OP.subtract)
            nc.vector.scalar_tensor_tensor(t1v, szv, -2.0, cpv, OP.mult, OP.add)
            nc.vector.tensor_tensor(fzv, t1v, t2v, OP.mult)
            # c = f + zf*wvz + fz*wvf
            nc.vector.tensor_tensor(t1v, zfv,
                                    _ap(wvz_r, 0, [[0, CH], [1, J]]), OP.mult)
            nc.vector.tensor_tensor(t2v, fzv,
                                    _ap(wvf_r, 0, [[0, CH], [1, J]]), OP.mult)
            nc.vector.tensor_tensor(t1v, t1v, t2v, OP.add)
            nc.vector.tensor_tensor(ccv, fv, t1v, OP.add)

        if limit == 'p2b':
            return
        # ------------- P3: suffix products -------------
        nc.vector.memset(_ap(sfx, (s_len - 1) * J, [[1, J]]), 1.0)
        zcol = cons.tile([128, 1], F32, tag="zcol")
        nc.vector.memset(zcol, 0.0)
        for j in range(J):
            nc.vector.tensor_tensor_scan(
                _ap(sfx, (s_len - 2) * J + j, [[-J, s_len - 1]]),
                _ap(cc, (s_len - 1) * J + j, [[-J, s_len - 1]]),
                _ap(zcol, 0, [[0, s_len - 1]]),
                1.0, OP.mult, OP.add)
        pfull = cons.tile([128, J], F32, tag="pfull")
        nc.vector.tensor_tensor(pfull, _ap(sfx, 0, [[1, J]]),
                                _ap(cc, 0, [[1, J]]), OP.mult)

        # ------------- G = suffix * zf / fz (in place) -------------
        nc.vector.tensor_tensor(zf[:], zf[:], sfx[:], OP.mult)
        nc.vector.tensor_tensor(fz[:], fz[:], sfx[:], OP.mult)

        if limit == 'p3':
            return
        # ------------- small-state sums -------------
        red = [[1, J], [J, s_len]]
        bzs = cons.tile([128, J], F32, tag="bzs")
        bfs = cons.tile([128, J], F32, tag="bfs")
        nc.vector.tensor_reduce(bzs, _ap(zf, 0, red), mybir.AxisListType.X,
                                OP.add)
        nc.vector.tensor_reduce(bfs, _ap(fz, 0, red), mybir.AxisListType.X,
                                OP.add)

        wzs = cons.tile([128, J], F32, tag="wzs")
        wfs = cons.tile([128, J], F32, tag="wfs")
        for st, g_t, acc_t in ((0, zf, wzs), (1, fz, wfs)):
            parts = []
            for chq in range(NCH):
                qt = b2_p.tile([128, CH * J], F32, tag="t1")
                nc.vector.tensor_tensor(
                    qt, _ap(g_t, chq * CH * J, [[1, CH * J]]),
                    _ap(cp, chq * CH * J, [[1, CH * J]]), OP.mult)
                pt = cons.tile([128, J], F32, tag=f"qp{st}_{chq}")
                nc.vector.tensor_reduce(
                    pt, _ap(qt, 0, [[1, J], [J, CH]]),
                    mybir.AxisListType.X, OP.add)
                parts.append(pt)
            nc.vector.tensor_tensor(acc_t, parts[0], parts[1], OP.add)
            for pt in parts[2:]:
                nc.vector.tensor_tensor(acc_t, acc_t, pt, OP.add)

        def store_bh(src, name):
            nc.sync.dma_start(out=io[name][:, :],
                              in_=_ap(src, 0, [[1, J]]))

        tmp1 = cons.tile([128, J], F32, tag="tmp1")
        for sum_t, b_t, s0_t, name in ((wzs, bzs, wz0_t, "wzn"),
                                       (wfs, bfs, wf0_t, "wfn")):
            nc.vector.tensor_tensor(tmp1, sum_t, b_t, OP.subtract)
            nc.vector.tensor_tensor(sum_t, s0_t, pfull, OP.mult)
            nc.vector.tensor_tensor(sum_t, sum_t, tmp1, OP.add)
            store_bh(sum_t, name)
        for b_t, s0_t, name in ((bzs, bz0_t, "bzn"), (bfs, bf0_t, "bfn")):
            nc.vector.tensor_tensor(tmp1, s0_t, pfull, OP.mult)
            nc.vector.tensor_tensor(b_t, b_t, tmp1, OP.add)
            store_bh(b_t, name)

        if limit == 'sums':
            return
        # ------------- P4: transpose G -------------
        gtz = big.tile([128, BL * SC * 512], F32, tag="xT")     # reuse slot
        gtf = big.tile([128, BL * SC * 512], F32, tag="pzfb")   # reuse slot
        for g_t, gt_t in ((zf, gtz), (fz, gtf)):
            for b in range(BL):
                for kt in range(SC):
                    for hh in range(HH):
                        tp = ppT.tile([128, 128], F32, tag="tp")
                        nc.tensor.transpose(
                            tp,
                            _ap(g_t, (kt * 128) * J + b * HH + hh, [[J, 128]]),
                            idn[:, :])
                        nc.scalar.activation(
                            _ap(gt_t, b * SC * 512 + kt * 512 + hh * 128,
                                [[1, 128]]),
                            tp[:, :], AF.Copy)

        if limit == 'p4':
            return
        # ------------- P5: state matmuls + combine + writeback -------------
        for st, gt_t, s0n, outn in ((0, gtz, "z0", "zout"),
                                    (1, gtf, "f0", "fout")):
            for b in range(BL):
                for hh in range(HH):
                    ps = pp5.tile([128, 512], F32, tag="pp5")
                    for kt in range(SC):
                        nc.tensor.matmul(
                            ps,
                            _ap(gt_t, b * SC * 512 + kt * 512 + hh * 128,
                                [[1, 128]]),
                            _ap(xnat, kt * BL * D + b * D, [[1, D]]),
                            start=(kt == 0), stop=(kt == SC - 1))
                    s0t = str_p.tile([128, 512], F32, tag="s0")
                    nc.sync.dma_start(
                        out=s0t,
                        in_=_dap(io[s0n], (b * H + hh * 128) * D, [D, 128],
                                 [[1, D]]))
                    sct = str_p.tile([128, 512], F32, tag="sc5")
                    nc.scalar.activation(
                        sct, s0t, AF.Copy, bias=0.0,
                        scale=pfull[:, b * HH + hh:b * HH + hh + 1])
                    ot = out_p.tile([128, 512], F32, tag="ot")
                    nc.vector.tensor_tensor(ot, sct, ps[:, :], OP.add)
                    nc.sync.dma_start(
                        out=_dap(io[outn], (b * H + hh * 128) * D, [D, 128],
                                 [[1, D]]),
                        in_=ot)


# ======================= host side =======================
_CACHE = {}


def _get_compiled(s_len=S):
    if s_len not in _CACHE:
        nc = bacc.Bacc("TRN2", target_bir_lowering=False, debug=False,
                       num_devices=NCORES)
        _CACHE[s_len] = build_kernel(nc, s_len)
    return _CACHE[s_len]


def make_in_maps(x, hidden_prev, Z_state, F_state, wz_state, wf_state,
                 bz_state, bf_state, wm_z, wm_f, wv_z, wv_f, bias_z, bias_f):
    x = np.ascontiguousarray(np.asarray(x, dtype=np.float32))
    ident = np.eye(128, dtype=np.float32)
    cnt = np.ascontiguousarray
    asn = lambda a: np.asarray(a, dtype=np.float32)
    in_maps = []
    for c in range(NCORES):
        bs = slice(c * BL, (c + 1) * BL)
        in_maps.append({
            "x": cnt(x[:, bs, :]),
            "hp": cnt(asn(hidden_prev)[0, bs]),
            "z0": cnt(asn(Z_state)[0, bs]),
            "f0": cnt(asn(F_state)[0, bs]),
            "wz0": cnt(asn(wz_state)[0, bs]),
            "wf0": cnt(asn(wf_state)[0, bs]),
            "bz0": cnt(asn(bz_state)[0, bs]),
            "bf0": cnt(asn(bf_state)[0, bs]),
            "wmz": cnt(asn(wm_z)),
            "wmf": cnt(asn(wm_f)),
            "wvz": cnt(asn(wv_z)),
            "wvf": cnt(asn(wv_f)),
            "bsz": cnt(asn(bias_z)),
            "bsf": cnt(asn(bias_f)),
            "ident": ident,
        })
    return in_maps


def assemble_outputs(rs):
    cells = np.concatenate([rs[c]["cells"] for c in range(NCORES)], axis=1)
    zn = np.concatenate([rs[c]["zout"] for c in range(NCORES)], axis=0)[None]
    fn = np.concatenate([rs[c]["fout"] for c in range(NCORES)], axis=0)[None]
    wzn = np.concatenate([rs[c]["wzn"] for c in range(NCORES)], axis=0)[None]
    wfn = np.concatenate([rs[c]["wfn"] for c in range(NCORES)], axis=0)[None]
    bzn = np.concatenate([rs[c]["bzn"] for c in range(NCORES)], axis=0)[None]
    bfn = np.concatenate([rs[c]["bfn"] for c in range(NCORES)], axis=0)[None]
    new_cell = cells[-1][None]
    return (cells, new_cell, zn, fn, wzn, wfn, bzn, bfn)


def kernel(**inputs):
    nc = _get_compiled(S)
    in_maps = make_in_maps(**inputs)
    res = run_bass_kernel_spmd(nc, in_maps, list(range(NCORES)))
    return assemble_outputs(res.results)
